# revision 6
# baseline (speedup 1.0000x reference)
"""Bass/Trainium2 kernel for nn_BernoulliMixture.

Reference computation (fp32):
    h = leaky_relu(x @ W_i2h^T + b_i2h)              [4096, 1024]
    z = softmax(h @ W_h2z^T + b_h2z)                 [4096, 32]
    d = sigmoid((h @ W_h2d^T + b_h2d) -> [.., 32, 784])
    out = einsum('tk,tko->to', z, d)                 [4096, 784]

Sharding (8 cores, SPMD): 8 token groups; each core handles 512 tokens
and all 32 components.

The dominant h2d matmul (1024 x 25088 per token) runs in fp8 e4m3 with
perf_mode=DoubleRow (2 fp8 weights per PE cell, 256-deep contraction per
pass) for ~2x the bf16 PE rate.  Scaling: W_i2h is pre-scaled 16x on the
host so h is carried at 16x (fp8-friendly range); w_h2d is pre-scaled
16x and stored e4m3; sigmoid reads PSUM directly with scale=1/256.
w_h2z is pre-scaled 1/16 so the softmax logits stay exact.  Numerically
simulated max rel err vs fp64: ~1.5e-2 (gate 2e-2).

Both bias adds are folded into the PE as rank-1 matmul accumulations
(ones x bias_row), so the per-element tail work is exactly one op per
engine stage:
  PE:      d-logits (DoubleRow fp8) + bias rank-1          -> PSUM
  ScalarE: ds = sigmoid(psum * 1/256)                       PSUM->SBUF
  DVE:     U += esc_k * ds  (per-partition-scalar stt)      SBUF
Phase H similarly: PE adds the i2h bias, one DVE stt computes
16*leaky_relu as max(0.01*ph, ph) straight into fp8, and GpSimd
dequantizes that to bf16 for the (tiny) softmax matmul.
"""

import os
from contextlib import ExitStack

import numpy as np

# ---------------------------------------------------------------------------
# problem constants (hardcoded; kernel.py must be self-contained)
B, L, IN, HID, K, O = 4, 1024, 512, 1024, 32, 784
N_CORES = 8
TOK_GROUPS = 8          # token-parallel
T = (B * L) // TOK_GROUPS          # 512 tokens per core
R = K * O                           # 25088 d-columns per core
# d-matmul psum windows: 1024-wide (2 PSUM banks) for the bulk, tapered at
# the end so the PE->DVE pipeline drains with less backlog
WIN_PLAN = [1024] * (R // 1024 - 2)
_rest = R - sum(WIN_PLAN)
while _rest > 512:
    WIN_PLAN.append(512)
    _rest -= 512
while _rest:
    WIN_PLAN.append(256)
    _rest -= 256
assert sum(WIN_PLAN) == R
WIN_OFF = [sum(WIN_PLAN[:i]) for i in range(len(WIN_PLAN))]
N_WIN = len(WIN_PLAN)
TCHUNKS = T // 128                  # 4
JC = HID // 128                     # 8 contraction chunks of h
JC2 = JC // 2                       # 4 DoubleRow pair-chunks
IC = IN // 128                      # 4 contraction chunks of x
HSCALE = 16.0                       # h carried at 16x for fp8 range
WSCALE = 16.0                       # w_h2d carried at 16x for fp8 range
DSCALE = 1.0 / (HSCALE * WSCALE)    # psum -> logit correction

_PROGRAM = None


def _install_drain_patch():
    """This image's walrus accepts at most ONE sync wait on CTRL-class
    instructions (Drain/NoOp). Stock Tile puts one wait per outstanding
    semaphore on the kernel-tail drain; split the extras into a chain of
    single-wait NOPs."""
    import concourse.tile as tile
    import concourse.mybir as mybir

    if getattr(tile.TileContext, "_drain_patch_installed", False):
        return

    def _drain_and_barrier(self, tick_clock, wait_clock):
        nc = self.nc
        drain_inst = nc.sync.drain()
        wait_clock.add_sem_waits(
            drain_inst.ins, tile.ScopedClock({None: tick_clock.global_clock})
        )
        si = drain_inst.ins.sync_info
        waits = list(si.on_wait or []) if si is not None else []
        if len(waits) > 1:
            si.on_wait = waits[:1]
            for w in waits[1:]:
                nop = nc.sync.nop()
                nop.ins.sync_info = mybir.SyncInfo(on_wait=[w], on_update=[])

        nc.all_engine_barrier()
        assert self.sems is not None
        popped = nc._tile_sem_poison_stack.pop()
        assert popped is self._sem_poison
        nc.clear_and_free_semaphores(list(self.sems.allocated().values()))
        nc.all_engine_barrier()

    tile.TileContext._drain_and_barrier = _drain_and_barrier
    tile.TileContext._drain_patch_installed = True


def _legalize_waits(nc):
    """This image's walrus accepts at most ONE sync wait per instruction.
    Hoist extra waits into preceding single-wait NOPs on the same engine
    (engines execute their stream in order, so a prior NOP-wait gates the
    instruction identically)."""
    import concourse.mybir as mybir

    n = 0
    for bass_bb in nc.bb_map.values():
        insts = bass_bb.bb.instructions
        i = 0
        while i < len(insts):
            inst = insts[i]
            si = inst.sync_info
            waits = list(si.on_wait) if si is not None and si.on_wait else []
            if len(waits) > 1:
                for w in waits[:-1]:
                    nop = mybir.InstNoOp(
                        name=f"waitnop_{n}", engine=inst.engine, ins=[], outs=[],
                        sync_info=mybir.SyncInfo(on_wait=[w], on_update=[]),
                    )
                    n += 1
                    insts.insert(i, nop)
                    i += 1
                si.on_wait = waits[-1:]
            i += 1
    return n


def _d_segments(w0, w1):
    """(kk, s0, s1) pieces of dram-column range [w0, w1) split at component
    boundaries (784 columns per component)."""
    segs = []
    for kk in range(w0 // O, (w1 - 1) // O + 1):
        s0, s1 = max(w0, kk * O), min(w1, (kk + 1) * O)
        segs.append((kk, s0, s1))
    return segs


def _build_program():
    import concourse.bass as bass
    import concourse.mybir as mybir
    import concourse.tile as tile

    _install_drain_patch()
    f32 = mybir.dt.float32
    bf16 = mybir.dt.bfloat16
    f8 = mybir.dt.float8e4
    AF = mybir.ActivationFunctionType
    ALU = mybir.AluOpType
    DR = mybir.MatmulPerfMode.DoubleRow

    nc = bass.Bass("TRN2", target_bir_lowering=False, debug=False,
                   num_devices=N_CORES)

    d_xT = nc.dram_tensor("xT", [IC, 128, T], bf16, kind="ExternalInput").ap()
    d_wi2hT = nc.dram_tensor("wi2hT", [IC, 128, HID], bf16,
                             kind="ExternalInput").ap()
    d_bi16 = nc.dram_tensor("bi16", [1, HID], bf16, kind="ExternalInput").ap()
    d_wzT = nc.dram_tensor("wzT", [128, JC, K], bf16, kind="ExternalInput").ap()
    d_bz = nc.dram_tensor("bz", [1, K], f32, kind="ExternalInput").ap()
    d_wdT = nc.dram_tensor("wdT", [128, JC, R], f8, kind="ExternalInput").ap()
    d_bd256 = nc.dram_tensor("bd256", [1, R], bf16, kind="ExternalInput").ap()
    d_out = nc.dram_tensor("out", [T, O], f32, kind="ExternalOutput").ap()

    with tile.TileContext(nc) as tc:
        with (
            tc.tile_pool(name="consts", bufs=1) as consts,
            tc.tile_pool(name="hpool", bufs=1) as hpool,
            tc.tile_pool(name="upool", bufs=1) as upool,
            tc.tile_pool(name="epool", bufs=1) as epool,
            tc.tile_pool(name="tmp", bufs=2) as tmp,
        ):

            # ---- phase H: h8[j, t] = fp8(16*leaky_relu(x W^T + b)) ---------
            h_sb = [hpool.tile([128, T], bf16, tag=f"h{j}", name=f"h{j}")
                    for j in range(JC)]
            h8_sb = hpool.tile([128, JC, T], f8, tag="h8", name="h8")
            hzctx = ExitStack()
            hz_psum = hzctx.enter_context(
                tc.tile_pool(name="hz_psum", bufs=4, space="PSUM"))
            esc_sb = [None] * TCHUNKS
            dctx = ExitStack()
            wslab_pool = dctx.enter_context(tc.tile_pool(name="wslab", bufs=6))
            dtmp = dctx.enter_context(tc.tile_pool(name="dtmp", bufs=3))

            def load_slabs(w):
                w0 = WIN_OFF[w]
                win = WIN_PLAN[w]
                wsls = []
                for sub in range(0, win, 512):
                    sw = min(512, win - sub)
                    wsl = wslab_pool.tile([128, JC, sw], f8, tag="w",
                                          name=f"wsl{w}_{sub}")
                    for ja in range(0, JC, 2):
                        nc.sync.dma_start(
                            wsl[:, ja:ja + 2, :],
                            d_wdT[:, ja:ja + 2, w0 + sub:w0 + sub + sw])
                    wsls.append((sub, sw, wsl))
                return wsls

            with (
                tc.tile_pool(name="xw", bufs=1) as xw,
            ):
                x_sb, wi_sb = [], []
                for i in range(IC):
                    xt = xw.tile([128, T], bf16, tag=f"x{i}", name=f"x{i}")
                    x_sb.append(xt)
                    wt = xw.tile([128, HID], bf16, tag=f"wi{i}", name=f"wi{i}")
                    wi_sb.append(wt)
                # split the loads so the first matmul's operands land first
                # (one dma_start = one HW queue; fine pieces spread queues)
                for i in range(IC):
                    nc.sync.dma_start(wi_sb[i][:, 0:128], d_wi2hT[i][:, 0:128])
                    nc.scalar.dma_start(x_sb[i][:, 0:256], d_xT[i][:, 0:256])
                    nc.sync.dma_start(x_sb[i][:, 256:512], d_xT[i][:, 256:512])
                # constants ride the scalar-engine DMA queues, off the
                # critical SP dispatch path
                bi16_sb = consts.tile([1, HID], bf16)
                nc.scalar.dma_start(bi16_sb[:], d_bi16[:])
                wz_sb = consts.tile([128, JC, K], bf16)
                nc.scalar.dma_start(wz_sb[:], d_wzT[:])
                bz_sb = consts.tile([1, K], f32)
                nc.scalar.dma_start(bz_sb[:], d_bz[:])
                bd_sb = consts.tile([1, R], bf16)
                nc.scalar.dma_start(bd_sb[:, 0:R // 2], d_bd256[:, 0:R // 2])
                nc.scalar.dma_start(bd_sb[:, R // 2:R], d_bd256[:, R // 2:R])
                ones_sb = consts.tile([1, 128], f32)
                nc.vector.memset(ones_sb[:], 1.0)
                onesb_sb = consts.tile([1, 128], bf16)
                nc.vector.memset(onesb_sb[:], 1.0)
                onest_sb = consts.tile([1, 512], bf16)
                nc.vector.memset(onest_sb[:], 1.0)
                u_sb = []
                for t in range(TCHUNKS):
                    u = upool.tile([128, O], f32, tag=f"u{t}", name=f"u{t}")
                    nc.vector.memset(u[:], 0.0)
                    u_sb.append(u)
                for i in range(IC):
                    for n4, (c0, c1) in enumerate(((128, 512), (512, 768),
                                                   (768, HID))):
                        eng = nc.scalar if n4 % 2 else nc.sync
                        eng.dma_start(wi_sb[i][:, c0:c1], d_wi2hT[i][:, c0:c1])
                    if T > 512:
                        nc.scalar.dma_start(x_sb[i][:, 512:T],
                                            d_xT[i][:, 512:T])
                preloaded = {w: load_slabs(w) for w in range(3)}

                # H and Z interleaved: after each 512-token half of h is
                # done, immediately compute that half's softmax numerators
                for tw in range(T // 512):
                    for j in range(JC):
                        ph = hz_psum.tile([128, 512], f32, tag="ph")
                        # bias via rank-1: b_row[j-chunk] x ones_tokens
                        nc.tensor.matmul(
                            ph[:],
                            lhsT=bi16_sb[:, j * 128:(j + 1) * 128],
                            rhs=onest_sb[:],
                            start=True,
                            stop=False,
                        )
                        for i in range(IC):
                            nc.tensor.matmul(
                                ph[:],
                                lhsT=wi_sb[i][:, j * 128:(j + 1) * 128],
                                rhs=x_sb[i][:, tw * 512:(tw + 1) * 512],
                                start=False,
                                stop=(i == IC - 1),
                            )
                        # 16*leaky_relu = 0.01*ph + 0.99*relu(ph); split so the
                        # DVE stt reads only one PSUM operand (HW restriction)
                        r1 = xw.tile([128, 512], f32, tag="r1", bufs=2,
                                     name=f"r1_{tw}_{j}")
                        nc.scalar.activation(r1[:], ph[:], AF.Relu, scale=0.99)
                        nc.vector.scalar_tensor_tensor(
                            out=h8_sb[:, j, tw * 512:(tw + 1) * 512],
                            in0=ph[:], scalar=0.01, in1=r1[:],
                            op0=ALU.mult, op1=ALU.add,
                        )
                        # dequantized bf16 copy for the softmax matmul
                        # (ScalarE: the gpsimd fp8 path measures ~14ns/elem)
                        nc.scalar.activation(
                            h_sb[j][:, tw * 512:(tw + 1) * 512],
                            h8_sb[:, j, tw * 512:(tw + 1) * 512],
                            AF.Copy,
                        )
                    for t in range(tw * 4, tw * 4 + 4):
                        pz = hz_psum.tile([128, K], f32, tag="pz",
                                          name=f"pz{t}")
                        for j in range(JC):
                            nc.tensor.matmul(
                                pz[:],
                                lhsT=h_sb[j][:, t * 128:(t + 1) * 128],
                                rhs=wz_sb[:, j, :],
                                start=(j == 0),
                                stop=False,
                            )
                        # + b_h2z via rank-1 update: ones[t] x bz
                        nc.tensor.matmul(
                            pz[:],
                            lhsT=ones_sb[:],
                            rhs=bz_sb[:],
                            start=False,
                            stop=True,
                        )
                        e_t = epool.tile([128, K], f32, tag=f"e{t}",
                                         name=f"e{t}")
                        s_t = tmp.tile([128, 1], f32, tag="s", name=f"s{t}")
                        nc.scalar.activation(e_t[:], pz[:], AF.Exp,
                                             accum_out=s_t[:])
                        sinv = tmp.tile([128, 1], f32, tag="sinv",
                                        name=f"sinv{t}")
                        nc.vector.reciprocal(sinv[:], s_t[:])
                        esc = epool.tile([128, K], f32, tag=f"esc{t}",
                                         name=f"esc{t}")
                        nc.vector.tensor_scalar(esc[:], e_t[:], sinv[:], None,
                                                ALU.mult)
                        esc_sb[t] = esc

            # ---- phase D: stream W shard, accumulate U ---------------------
            hzctx.close()
            pctx = ExitStack()
            d_psum = pctx.enter_context(
                tc.tile_pool(name="d_psum", bufs=4, space="PSUM"))
            for w in range(N_WIN):
                w0 = WIN_OFF[w]
                win = WIN_PLAN[w]
                w1 = w0 + win
                wsls = preloaded.pop(w) if w in preloaded else load_slabs(w)
                segs = _d_segments(w0, w1)
                t_order = range(TCHUNKS)
                if w == N_WIN - 1:
                    t_order = reversed(range(TCHUNKS))
                for t in t_order:
                    pd = d_psum.tile([128, win], f32, tag="pd", name=f"pd{w}_{t}")
                    # per 512-sub: bias rank-1 starts the psum group, then
                    # the 4 DoubleRow pair-matmuls accumulate.  j2 outer /
                    # sub inner: both subs reuse the same stationary h tile.
                    for sub, sw, wsl in wsls:
                        nc.tensor.matmul(
                            pd[:, sub:sub + sw],
                            lhsT=onesb_sb[:],
                            rhs=bd_sb[:, w0 + sub:w0 + sub + sw],
                            start=True,
                            stop=False,
                        )
                    for j2 in range(JC2):
                        for sub, sw, wsl in wsls:
                            nc.tensor.matmul(
                                pd[:, sub:sub + sw],
                                lhsT=h8_sb[:, 2 * j2:2 * j2 + 2,
                                           t * 128:(t + 1) * 128],
                                rhs=wsl[:, 2 * j2:2 * j2 + 2, :],
                                start=False,
                                stop=(j2 == JC2 - 1),
                                perf_mode=DR,
                            )
                    ds = dtmp.tile([128, win], f32, tag="ds")
                    nc.scalar.activation(ds[:], pd[:], AF.Sigmoid,
                                         scale=DSCALE)
                    for kk, s0, s1 in segs:
                        nc.vector.scalar_tensor_tensor(
                            out=u_sb[t][:, s0 - kk * O:s1 - kk * O],
                            in0=ds[:, s0 - w0:s1 - w0],
                            scalar=esc_sb[t][:, kk:kk + 1],
                            in1=u_sb[t][:, s0 - kk * O:s1 - kk * O],
                            op0=ALU.mult, op1=ALU.add,
                        )

            for t in reversed(range(TCHUNKS)):
                nc.scalar.dma_start(d_out[t * 128:(t + 1) * 128, 0:392],
                                  u_sb[t][:, 0:392])
                nc.scalar.dma_start(d_out[t * 128:(t + 1) * 128, 392:O],
                                  u_sb[t][:, 392:O])
            pctx.close()
            dctx.close()

    _legalize_waits(nc)
    return nc


def _get_program():
    global _PROGRAM
    if _PROGRAM is None:
        _PROGRAM = _build_program()
    return _PROGRAM


def _prep_inputs(input, w_i2h, b_i2h, w_h2z, b_h2z, w_h2d, b_h2d):
    """Build the 8 per-core in_maps (host-side transposes/shards)."""
    import ml_dtypes
    x_flat = np.ascontiguousarray(input.reshape(B * L, IN).astype(np.float32))
    # W_i2h pre-scaled 16x (exact in bf16): h is carried at 16x everywhere
    wi2hT = np.ascontiguousarray(
        HSCALE * w_i2h.astype(np.float32).T.reshape(IC, 128, HID)
    ).astype(ml_dtypes.bfloat16)
    bi16 = np.ascontiguousarray(
        (HSCALE * b_i2h.astype(np.float32)).reshape(1, HID)
    ).astype(ml_dtypes.bfloat16)

    # w_h2z pre-scaled 1/16: (16h) @ (wz/16) keeps softmax logits exact
    wzT_full = w_h2z.astype(np.float32).T / HSCALE   # [HID, K]
    bz = np.ascontiguousarray(b_h2z.astype(np.float32).reshape(1, K))
    wz = np.ascontiguousarray(
        wzT_full.reshape(JC, 128, K).transpose(1, 0, 2)
    ).astype(ml_dtypes.bfloat16)

    # w_h2d at 16x in e4m3 (TRN FP8_EXP4: same bits as OCP e4m3 below
    # |240|; 16*w stays ~N(0, 0.25) so everything is in normal range)
    wdT_full = w_h2d.astype(np.float32).T            # [HID, R]
    wd = np.ascontiguousarray(
        (WSCALE * wdT_full).reshape(JC, 128, R).transpose(1, 0, 2)
    ).astype(ml_dtypes.float8_e4m3)
    # d-bias at 256x (compensated by sigmoid's 1/256 input scale)
    bd256 = np.ascontiguousarray(
        (b_h2d.astype(np.float32) / DSCALE).reshape(1, R)
    ).astype(ml_dtypes.bfloat16)

    in_maps = []
    for core in range(N_CORES):
        xT = np.ascontiguousarray(
            x_flat[core * T:(core + 1) * T, :].T.reshape(IC, 128, T)
        ).astype(ml_dtypes.bfloat16)
        in_maps.append({
            "xT": xT, "wi2hT": wi2hT, "bi16": bi16,
            "wzT": wz, "bz": bz, "wdT": wd, "bd256": bd256,
        })
    return in_maps


LAST_RESULT = None


def kernel(**inputs):
    from concourse.bass_utils import run_bass_kernel_spmd

    global LAST_RESULT
    nc = _get_program()
    in_maps = _prep_inputs(**inputs)
    trace = bool(os.environ.get("BASS_KERNEL_TRACE"))
    if trace:
        try:
            _install_profile_shim()
        except Exception as e:  # degrade to untraced run
            print(f"profile shim unavailable ({e}); running untraced")
            trace = False
    res = run_bass_kernel_spmd(nc, in_maps, list(range(N_CORES)), trace=trace)
    LAST_RESULT = res

    out = np.empty((B * L, O), dtype=np.float32)
    for tg in range(TOK_GROUPS):
        out[tg * T:(tg + 1) * T] = res.results[tg]["out"].astype(np.float32)
    return out.reshape(B, L, O)


def _install_profile_shim():
    """Register the NTFF profile hook concourse expects under axon (the
    image's antenv lacks axon_hooks) and stub the artifact upload."""
    import sys
    import types

    if "antenv.axon_hooks" not in sys.modules:
        from trn_agent_boot.trn_boot import _ntff_profile_via_ctypes

        hook = _ntff_profile_via_ctypes("/opt/axon/libaxon_pjrt.so")
        m = types.ModuleType("antenv.axon_hooks")
        m.get_axon_ntff_profile_hook = lambda: hook
        m.set_axon_ntff_profile_hook = lambda h: None
        sys.modules["antenv.axon_hooks"] = m

    import concourse.bass_utils as bu

    bu.upload_artifacts = lambda tmpdir: f"local://{tmpdir}"


# revision 10
# speedup vs baseline: 1.2413x; 1.2413x over previous
"""Bass/Trainium2 kernel for nn_BernoulliMixture.

Reference computation (fp32):
    h = leaky_relu(x @ W_i2h^T + b_i2h)              [4096, 1024]
    z = softmax(h @ W_h2z^T + b_h2z)                 [4096, 32]
    d = sigmoid((h @ W_h2d^T + b_h2d) -> [.., 32, 784])
    out = einsum('tk,tko->to', z, d)                 [4096, 784]

Sharding (8 cores, SPMD): 8 token groups; each core handles 512 tokens
and all 32 components.

The dominant h2d matmul (1024 x 25088 per token) runs in fp8 e4m3 with
perf_mode=DoubleRow (2 fp8 weights per PE cell, 256-deep contraction per
pass): measured 259 ns per 512-col matmul vs 225 ns bf16 at half the
instruction count -> d-phase PE ~203 us vs ~350 us bf16.  Scaling:
W_i2h is pre-scaled 16x on the host so h is carried at 16x
(fp8-friendly range); w_h2d is pre-scaled 16x and stored e4m3; w_h2z is
pre-scaled 1/16 so the softmax logits stay exact (the z matmul consumes
the fp8 h directly - mixed fp8 x bf16 matmul is legal).  Numerically
simulated max rel err vs fp64: ~1.5e-2 (gate 2e-2, HW matches sim to
<1e-4).

The d-bias is split between engines to balance their busy time
(PE ~215 us of matmul vs DVE ~150 us of U-accumulation):
  - every 4th window: bias via PE rank-1 (ones x 256*b row), sigmoid
    reads PSUM directly with scale=1/256;
  - other windows: DVE stt db = psum*(1/256) + bias_slab, sigmoid reads
    SBUF.
Phase H folds the i2h bias into a PE rank-1 and computes
16*leaky_relu(ph) = 0.01*ph + 0.99*relu(ph) with one ScalarE op and one
DVE stt that writes e4m3 directly (single rounding).  hz/d PSUM pools
coexist (2+6 banks) so phase D starts without a pool-swap barrier.
"""

import os
from contextlib import ExitStack

import numpy as np

# ---------------------------------------------------------------------------
# problem constants (hardcoded; kernel.py must be self-contained)
B, L, IN, HID, K, O = 4, 1024, 512, 1024, 32, 784
N_CORES = 8
TOK_GROUPS = 8          # token-parallel
T = (B * L) // TOK_GROUPS          # 512 tokens per core
R = K * O                           # 25088 d-columns per core
WIN_PLAN = [1024] * (R // 1024) + [R % 1024]   # 24 x 1024 + 512
assert sum(WIN_PLAN) == R
WIN_OFF = [sum(WIN_PLAN[:i]) for i in range(len(WIN_PLAN))]
N_WIN = len(WIN_PLAN)
PE_BIAS_EVERY = 4                   # windows w%4==0 take the PE rank-1 bias
TCHUNKS = T // 128                  # 4
JC = HID // 128                     # 8 contraction chunks of h
JC2 = JC // 2                       # 4 DoubleRow pair-chunks
IC = IN // 128                      # 4 contraction chunks of x
HSCALE = 16.0                       # h carried at 16x for fp8 range
WSCALE = 16.0                       # w_h2d carried at 16x for fp8 range
DSCALE = 1.0 / (HSCALE * WSCALE)    # psum -> logit correction

_PROGRAM = None


def _install_drain_patch():
    """This image's walrus accepts at most ONE sync wait on CTRL-class
    instructions (Drain/NoOp). Stock Tile puts one wait per outstanding
    semaphore on the kernel-tail drain; split the extras into a chain of
    single-wait NOPs."""
    import concourse.tile as tile
    import concourse.mybir as mybir

    if getattr(tile.TileContext, "_drain_patch_installed", False):
        return

    def _drain_and_barrier(self, tick_clock, wait_clock):
        nc = self.nc
        drain_inst = nc.sync.drain()
        wait_clock.add_sem_waits(
            drain_inst.ins, tile.ScopedClock({None: tick_clock.global_clock})
        )
        si = drain_inst.ins.sync_info
        waits = list(si.on_wait or []) if si is not None else []
        if len(waits) > 1:
            si.on_wait = waits[:1]
            for w in waits[1:]:
                nop = nc.sync.nop()
                nop.ins.sync_info = mybir.SyncInfo(on_wait=[w], on_update=[])

        nc.all_engine_barrier()
        assert self.sems is not None
        popped = nc._tile_sem_poison_stack.pop()
        assert popped is self._sem_poison
        nc.clear_and_free_semaphores(list(self.sems.allocated().values()))
        nc.all_engine_barrier()

    tile.TileContext._drain_and_barrier = _drain_and_barrier
    tile.TileContext._drain_patch_installed = True


def _legalize_waits(nc):
    """This image's walrus accepts at most ONE sync wait per instruction.
    Hoist extra waits into preceding single-wait NOPs on the same engine
    (engines execute their stream in order, so a prior NOP-wait gates the
    instruction identically)."""
    import concourse.mybir as mybir

    n = 0
    for bass_bb in nc.bb_map.values():
        insts = bass_bb.bb.instructions
        i = 0
        while i < len(insts):
            inst = insts[i]
            si = inst.sync_info
            waits = list(si.on_wait) if si is not None and si.on_wait else []
            if len(waits) > 1:
                for w in waits[:-1]:
                    nop = mybir.InstNoOp(
                        name=f"waitnop_{n}", engine=inst.engine, ins=[], outs=[],
                        sync_info=mybir.SyncInfo(on_wait=[w], on_update=[]),
                    )
                    n += 1
                    insts.insert(i, nop)
                    i += 1
                si.on_wait = waits[-1:]
            i += 1
    return n


def _d_segments(w0, w1):
    """(kk, s0, s1) pieces of dram-column range [w0, w1) split at component
    boundaries (784 columns per component)."""
    segs = []
    for kk in range(w0 // O, (w1 - 1) // O + 1):
        s0, s1 = max(w0, kk * O), min(w1, (kk + 1) * O)
        segs.append((kk, s0, s1))
    return segs


def _build_program():
    import concourse.bass as bass
    import concourse.mybir as mybir
    import concourse.tile as tile

    _install_drain_patch()
    f32 = mybir.dt.float32
    bf16 = mybir.dt.bfloat16
    f8 = mybir.dt.float8e4
    AF = mybir.ActivationFunctionType
    ALU = mybir.AluOpType
    DR = mybir.MatmulPerfMode.DoubleRow

    nc = bass.Bass("TRN2", target_bir_lowering=False, debug=False,
                   num_devices=N_CORES)

    d_xT = nc.dram_tensor("xT", [IC, 128, T], bf16, kind="ExternalInput").ap()
    d_wi2hT = nc.dram_tensor("wi2hT", [IC, 128, HID], bf16,
                             kind="ExternalInput").ap()
    d_bi16 = nc.dram_tensor("bi16", [1, HID], bf16, kind="ExternalInput").ap()
    d_wzT = nc.dram_tensor("wzT", [128, JC, K], bf16, kind="ExternalInput").ap()
    d_bz = nc.dram_tensor("bz", [1, K], f32, kind="ExternalInput").ap()
    d_wdT = nc.dram_tensor("wdT", [128, JC, R], f8, kind="ExternalInput").ap()
    d_bd256 = nc.dram_tensor("bd256", [1, R], bf16, kind="ExternalInput").ap()
    d_bdb = nc.dram_tensor("bdb", [128, R], bf16, kind="ExternalInput").ap()
    d_out = nc.dram_tensor("out", [T, O], f32, kind="ExternalOutput").ap()

    with tile.TileContext(nc) as tc:
        with (
            tc.tile_pool(name="consts", bufs=1) as consts,
            tc.tile_pool(name="hpool", bufs=1) as hpool,
            tc.tile_pool(name="upool", bufs=1) as upool,
            tc.tile_pool(name="epool", bufs=1) as epool,
            tc.tile_pool(name="tmp", bufs=2) as tmp,
            tc.tile_pool(name="hz_psum", bufs=2, space="PSUM") as hz_psum,
            tc.tile_pool(name="z_psum", bufs=2, space="PSUM") as z_psum,
            tc.tile_pool(name="d_psum", bufs=2, space="PSUM") as d_psum,
            tc.tile_pool(name="wslab", bufs=6) as wslab_pool,
            tc.tile_pool(name="bslab", bufs=3) as bslab_pool,
            tc.tile_pool(name="dtmp", bufs=3) as dtmp,
        ):

            # ---- phase H: h8[j, t] = fp8(16*leaky_relu(x W^T + b)) ---------
            h8_sb = hpool.tile([128, JC, T], f8, tag="h8", name="h8")
            esc_sb = [None] * TCHUNKS

            def load_slabs(w):
                w0 = WIN_OFF[w]
                win = WIN_PLAN[w]
                wsls = []
                for sub in range(0, win, 512):
                    sw = min(512, win - sub)
                    wsl = wslab_pool.tile([128, JC, sw], f8, tag="w",
                                          name=f"wsl{w}_{sub}")
                    for ja in range(0, JC, 2):
                        nc.sync.dma_start(
                            wsl[:, ja:ja + 2, :],
                            d_wdT[:, ja:ja + 2, w0 + sub:w0 + sub + sw])
                    wsls.append((sub, sw, wsl))
                if w % PE_BIAS_EVERY:
                    bsl = bslab_pool.tile([128, win], bf16, tag="b",
                                          name=f"bsl{w}")
                    half = win // 2
                    nc.scalar.dma_start(bsl[:, 0:half], d_bdb[:, w0:w0 + half])
                    nc.scalar.dma_start(bsl[:, half:win],
                                        d_bdb[:, w0 + half:w0 + win])
                else:
                    bsl = None
                return wsls, bsl

            with (
                tc.tile_pool(name="xw", bufs=1) as xw,
            ):
                x_sb, wi_sb = [], []
                for i in range(IC):
                    xt = xw.tile([128, T], bf16, tag=f"x{i}", name=f"x{i}")
                    x_sb.append(xt)
                    wt = xw.tile([128, HID], bf16, tag=f"wi{i}", name=f"wi{i}")
                    wi_sb.append(wt)
                # split the loads so the first matmul's operands land first
                # (one dma_start = one HW queue; fine pieces spread queues)
                for i in range(IC):
                    nc.sync.dma_start(wi_sb[i][:, 0:128], d_wi2hT[i][:, 0:128])
                    nc.scalar.dma_start(x_sb[i][:, 0:256], d_xT[i][:, 0:256])
                    nc.sync.dma_start(x_sb[i][:, 256:512], d_xT[i][:, 256:512])
                # constants on the otherwise-idle vector/gpsimd DMA queues so
                # the first d-window rank-1 (needs bd) isn't gated on the
                # scalar queue backlog
                bi16_sb = consts.tile([1, HID], bf16)
                nc.gpsimd.dma_start(bi16_sb[:], d_bi16[:])
                bd_sb = consts.tile([1, R], bf16)
                nc.gpsimd.dma_start(bd_sb[:, 0:R // 2], d_bd256[:, 0:R // 2])
                nc.gpsimd.dma_start(bd_sb[:, R // 2:R], d_bd256[:, R // 2:R])
                wz_sb = consts.tile([128, JC, K], bf16)
                nc.gpsimd.dma_start(wz_sb[:], d_wzT[:])
                bz_sb = consts.tile([1, K], f32)
                nc.gpsimd.dma_start(bz_sb[:], d_bz[:])
                ones_sb = consts.tile([1, 128], f32)
                nc.vector.memset(ones_sb[:], 1.0)
                onesb_sb = consts.tile([1, 128], bf16)
                nc.vector.memset(onesb_sb[:], 1.0)
                onest_sb = consts.tile([1, 512], bf16)
                nc.vector.memset(onest_sb[:], 1.0)
                u_sb = []
                for t in range(TCHUNKS):
                    u = upool.tile([128, O], f32, tag=f"u{t}", name=f"u{t}")
                    nc.vector.memset(u[:], 0.0)
                    u_sb.append(u)
                for i in range(IC):
                    for n4, (c0, c1) in enumerate(((128, 512), (512, 768),
                                                   (768, HID))):
                        eng = nc.scalar if n4 % 2 else nc.sync
                        eng.dma_start(wi_sb[i][:, c0:c1], d_wi2hT[i][:, c0:c1])
                    if T > 512:
                        nc.scalar.dma_start(x_sb[i][:, 512:T],
                                            d_xT[i][:, 512:T])
                preloaded = {w: load_slabs(w) for w in range(3)}

                # H and Z interleaved: after each 512-token half of h is
                # done, immediately compute that half's softmax numerators
                for tw in range(T // 512):
                    for j in range(JC):
                        ph = hz_psum.tile([128, 512], f32, tag="ph")
                        # bias via rank-1: b_row[j-chunk] x ones_tokens
                        nc.tensor.matmul(
                            ph[:],
                            lhsT=bi16_sb[:, j * 128:(j + 1) * 128],
                            rhs=onest_sb[:],
                            start=True,
                            stop=False,
                        )
                        for i in range(IC):
                            nc.tensor.matmul(
                                ph[:],
                                lhsT=wi_sb[i][:, j * 128:(j + 1) * 128],
                                rhs=x_sb[i][:, tw * 512:(tw + 1) * 512],
                                start=False,
                                stop=(i == IC - 1),
                            )
                        # 16*leaky_relu = 0.01*ph + 0.99*relu(ph); split so the
                        # DVE stt reads only one PSUM operand (HW restriction)
                        r1 = xw.tile([128, 512], f32, tag="r1", bufs=2,
                                     name=f"r1_{tw}_{j}")
                        nc.scalar.activation(r1[:], ph[:], AF.Relu, scale=0.99)
                        nc.vector.scalar_tensor_tensor(
                            out=h8_sb[:, j, tw * 512:(tw + 1) * 512],
                            in0=ph[:], scalar=0.01, in1=r1[:],
                            op0=ALU.mult, op1=ALU.add,
                        )
                    for t in range(tw * 4, tw * 4 + 4):
                        pz = z_psum.tile([128, K], f32, tag="pz",
                                          name=f"pz{t}")
                        for j in range(JC):
                            # fp8 stationary x bf16 moving is legal
                            nc.tensor.matmul(
                                pz[:],
                                lhsT=h8_sb[:, j, t * 128:(t + 1) * 128],
                                rhs=wz_sb[:, j, :],
                                start=(j == 0),
                                stop=False,
                            )
                        # + b_h2z via rank-1 update: ones[t] x bz
                        nc.tensor.matmul(
                            pz[:],
                            lhsT=ones_sb[:],
                            rhs=bz_sb[:],
                            start=False,
                            stop=True,
                        )
                        e_t = epool.tile([128, K], f32, tag=f"e{t}",
                                         name=f"e{t}")
                        s_t = tmp.tile([128, 1], f32, tag="s", name=f"s{t}")
                        nc.scalar.activation(e_t[:], pz[:], AF.Exp,
                                             accum_out=s_t[:])
                        sinv = tmp.tile([128, 1], f32, tag="sinv",
                                        name=f"sinv{t}")
                        nc.vector.reciprocal(sinv[:], s_t[:])
                        esc = epool.tile([128, K], f32, tag=f"esc{t}",
                                         name=f"esc{t}")
                        nc.vector.tensor_scalar(esc[:], e_t[:], sinv[:], None,
                                                ALU.mult)
                        esc_sb[t] = esc

            # ---- phase D: stream W shard, accumulate U ---------------------
            for w in range(N_WIN):
                w0 = WIN_OFF[w]
                win = WIN_PLAN[w]
                w1 = w0 + win
                wsls, bsl = preloaded.pop(w) if w in preloaded else load_slabs(w)
                segs = _d_segments(w0, w1)
                pe_bias = w % PE_BIAS_EVERY == 0
                t_order = range(TCHUNKS)
                if w == N_WIN - 1:
                    t_order = reversed(range(TCHUNKS))
                for t in t_order:
                    pd = d_psum.tile([128, win], f32, tag="pd", name=f"pd{w}_{t}")
                    # j2 outer / sub inner: both 512-subs reuse the same
                    # stationary h tile
                    if pe_bias:
                        for sub, sw, wsl in wsls:
                            nc.tensor.matmul(
                                pd[:, sub:sub + sw],
                                lhsT=onesb_sb[:],
                                rhs=bd_sb[:, w0 + sub:w0 + sub + sw],
                                start=True,
                                stop=False,
                            )
                    for j2 in range(JC2):
                        for sub, sw, wsl in wsls:
                            nc.tensor.matmul(
                                pd[:, sub:sub + sw],
                                lhsT=h8_sb[:, 2 * j2:2 * j2 + 2,
                                           t * 128:(t + 1) * 128],
                                rhs=wsl[:, 2 * j2:2 * j2 + 2, :],
                                start=(not pe_bias and j2 == 0),
                                stop=(j2 == JC2 - 1),
                                perf_mode=DR,
                            )
                    ds = dtmp.tile([128, win], f32, tag="ds")
                    if pe_bias:
                        nc.scalar.activation(ds[:], pd[:], AF.Sigmoid,
                                             scale=DSCALE)
                    else:
                        db = dtmp.tile([128, win], f32, tag="db")
                        nc.vector.scalar_tensor_tensor(
                            out=db[:], in0=pd[:], scalar=DSCALE, in1=bsl[:],
                            op0=ALU.mult, op1=ALU.add,
                        )
                        nc.scalar.activation(ds[:], db[:], AF.Sigmoid)
                    for kk, s0, s1 in segs:
                        nc.vector.scalar_tensor_tensor(
                            out=u_sb[t][:, s0 - kk * O:s1 - kk * O],
                            in0=ds[:, s0 - w0:s1 - w0],
                            scalar=esc_sb[t][:, kk:kk + 1],
                            in1=u_sb[t][:, s0 - kk * O:s1 - kk * O],
                            op0=ALU.mult, op1=ALU.add,
                        )

            for t in reversed(range(TCHUNKS)):
                nc.scalar.dma_start(d_out[t * 128:(t + 1) * 128, 0:392],
                                  u_sb[t][:, 0:392])
                nc.scalar.dma_start(d_out[t * 128:(t + 1) * 128, 392:O],
                                  u_sb[t][:, 392:O])

    _legalize_waits(nc)
    return nc


def _get_program():
    global _PROGRAM
    if _PROGRAM is None:
        _PROGRAM = _build_program()
    return _PROGRAM


def _prep_inputs(input, w_i2h, b_i2h, w_h2z, b_h2z, w_h2d, b_h2d):
    """Build the 8 per-core in_maps (host-side transposes/shards)."""
    import ml_dtypes
    x_flat = np.ascontiguousarray(input.reshape(B * L, IN).astype(np.float32))
    # W_i2h pre-scaled 16x (exact in bf16): h is carried at 16x everywhere
    wi2hT = np.ascontiguousarray(
        HSCALE * w_i2h.astype(np.float32).T.reshape(IC, 128, HID)
    ).astype(ml_dtypes.bfloat16)
    bi16 = np.ascontiguousarray(
        (HSCALE * b_i2h.astype(np.float32)).reshape(1, HID)
    ).astype(ml_dtypes.bfloat16)

    # w_h2z pre-scaled 1/16: (16h) @ (wz/16) keeps softmax logits exact
    wzT_full = w_h2z.astype(np.float32).T / HSCALE   # [HID, K]
    bz = np.ascontiguousarray(b_h2z.astype(np.float32).reshape(1, K))
    wz = np.ascontiguousarray(
        wzT_full.reshape(JC, 128, K).transpose(1, 0, 2)
    ).astype(ml_dtypes.bfloat16)

    # w_h2d at 16x in e4m3 (TRN FP8_EXP4: same bits as OCP e4m3 below
    # |240|; 16*w stays ~N(0, 0.25) so everything is in normal range)
    wdT_full = w_h2d.astype(np.float32).T            # [HID, R]
    wd = np.ascontiguousarray(
        (WSCALE * wdT_full).reshape(JC, 128, R).transpose(1, 0, 2)
    ).astype(ml_dtypes.float8_e4m3)
    # d-bias at 256x (compensated by sigmoid's 1/256 input scale) for the
    # PE rank-1 path, and a broadcast unscaled copy for the DVE stt path
    bd_f32 = b_h2d.astype(np.float32)
    bd256 = np.ascontiguousarray(
        (bd_f32 / DSCALE).reshape(1, R)).astype(ml_dtypes.bfloat16)
    bdb = np.ascontiguousarray(np.broadcast_to(
        bd_f32.astype(ml_dtypes.bfloat16), (128, R)))

    in_maps = []
    for core in range(N_CORES):
        xT = np.ascontiguousarray(
            x_flat[core * T:(core + 1) * T, :].T.reshape(IC, 128, T)
        ).astype(ml_dtypes.bfloat16)
        in_maps.append({
            "xT": xT, "wi2hT": wi2hT, "bi16": bi16,
            "wzT": wz, "bz": bz, "wdT": wd, "bd256": bd256, "bdb": bdb,
        })
    return in_maps


LAST_RESULT = None


def kernel(**inputs):
    from concourse.bass_utils import run_bass_kernel_spmd

    global LAST_RESULT
    nc = _get_program()
    in_maps = _prep_inputs(**inputs)
    trace = bool(os.environ.get("BASS_KERNEL_TRACE"))
    if trace:
        try:
            _install_profile_shim()
        except Exception as e:  # degrade to untraced run
            print(f"profile shim unavailable ({e}); running untraced")
            trace = False
    res = run_bass_kernel_spmd(nc, in_maps, list(range(N_CORES)), trace=trace)
    LAST_RESULT = res

    out = np.empty((B * L, O), dtype=np.float32)
    for tg in range(TOK_GROUPS):
        out[tg * T:(tg + 1) * T] = res.results[tg]["out"].astype(np.float32)
    return out.reshape(B, L, O)


def _install_profile_shim():
    """Register the NTFF profile hook concourse expects under axon (the
    image's antenv lacks axon_hooks) and stub the artifact upload."""
    import sys
    import types

    if "antenv.axon_hooks" not in sys.modules:
        from trn_agent_boot.trn_boot import _ntff_profile_via_ctypes

        hook = _ntff_profile_via_ctypes("/opt/axon/libaxon_pjrt.so")
        m = types.ModuleType("antenv.axon_hooks")
        m.get_axon_ntff_profile_hook = lambda: hook
        m.set_axon_ntff_profile_hook = lambda h: None
        sys.modules["antenv.axon_hooks"] = m

    import concourse.bass_utils as bu

    bu.upload_artifacts = lambda tmpdir: f"local://{tmpdir}"


# revision 11
# speedup vs baseline: 1.2803x; 1.0313x over previous
"""Bass/Trainium2 kernel for nn_BernoulliMixture.

Reference computation (fp32):
    h = leaky_relu(x @ W_i2h^T + b_i2h)              [4096, 1024]
    z = softmax(h @ W_h2z^T + b_h2z)                 [4096, 32]
    d = sigmoid((h @ W_h2d^T + b_h2d) -> [.., 32, 784])
    out = einsum('tk,tko->to', z, d)                 [4096, 784]

Sharding (8 cores, SPMD): 8 token groups; each core handles 512 tokens
and all 32 components.

The dominant h2d matmul (1024 x 25088 per token) runs in fp8 e4m3 with
perf_mode=DoubleRow (2 fp8 weights per PE cell, 256-deep contraction per
pass): measured 259 ns per 512-col matmul vs 225 ns bf16 at half the
instruction count -> d-phase PE ~203 us vs ~350 us bf16.  Scaling:
W_i2h is pre-scaled 16x on the host so h is carried at 16x
(fp8-friendly range); w_h2d is pre-scaled 16x and stored e4m3; w_h2z is
pre-scaled 1/16 so the softmax logits stay exact (the z matmul consumes
the fp8 h directly - mixed fp8 x bf16 matmul is legal).  Numerically
simulated max rel err vs fp64: ~1.5e-2 (gate 2e-2, HW matches sim to
<1e-4).

The d-bias is split between engines to balance their busy time
(PE ~215 us of matmul vs DVE ~150 us of U-accumulation):
  - every 4th window: bias via PE rank-1 (ones x 256*b row), sigmoid
    reads PSUM directly with scale=1/256;
  - other windows: DVE stt db = psum*(1/256) + bias_slab, sigmoid reads
    SBUF.
Phase H folds the i2h bias into a PE rank-1 and computes
16*leaky_relu(ph) = 0.01*ph + 0.99*relu(ph) with one ScalarE op and one
DVE stt that writes e4m3 directly (single rounding).  hz/d PSUM pools
coexist (2+6 banks) so phase D starts without a pool-swap barrier.
"""

import os
from contextlib import ExitStack

import numpy as np

# ---------------------------------------------------------------------------
# problem constants (hardcoded; kernel.py must be self-contained)
B, L, IN, HID, K, O = 4, 1024, 512, 1024, 32, 784
N_CORES = 8
TOK_GROUPS = 8          # token-parallel
T = (B * L) // TOK_GROUPS          # 512 tokens per core
R = K * O                           # 25088 d-columns per core
WIN_PLAN = [1024] * (R // 1024) + [R % 1024]   # 24 x 1024 + 512
assert sum(WIN_PLAN) == R
WIN_OFF = [sum(WIN_PLAN[:i]) for i in range(len(WIN_PLAN))]
N_WIN = len(WIN_PLAN)
PE_BIAS_EVERY = 3                   # windows w%3==0 take the PE rank-1 bias
TCHUNKS = T // 128                  # 4
JC = HID // 128                     # 8 contraction chunks of h
JC2 = JC // 2                       # 4 DoubleRow pair-chunks
IC = IN // 128                      # 4 contraction chunks of x
HSCALE = 16.0                       # h carried at 16x for fp8 range
WSCALE = 16.0                       # w_h2d carried at 16x for fp8 range
DSCALE = 1.0 / (HSCALE * WSCALE)    # psum -> logit correction

_PROGRAM = None


def _install_drain_patch():
    """This image's walrus accepts at most ONE sync wait on CTRL-class
    instructions (Drain/NoOp). Stock Tile puts one wait per outstanding
    semaphore on the kernel-tail drain; split the extras into a chain of
    single-wait NOPs."""
    import concourse.tile as tile
    import concourse.mybir as mybir

    if getattr(tile.TileContext, "_drain_patch_installed", False):
        return

    def _drain_and_barrier(self, tick_clock, wait_clock):
        nc = self.nc
        drain_inst = nc.sync.drain()
        wait_clock.add_sem_waits(
            drain_inst.ins, tile.ScopedClock({None: tick_clock.global_clock})
        )
        si = drain_inst.ins.sync_info
        waits = list(si.on_wait or []) if si is not None else []
        if len(waits) > 1:
            si.on_wait = waits[:1]
            for w in waits[1:]:
                nop = nc.sync.nop()
                nop.ins.sync_info = mybir.SyncInfo(on_wait=[w], on_update=[])

        nc.all_engine_barrier()
        assert self.sems is not None
        popped = nc._tile_sem_poison_stack.pop()
        assert popped is self._sem_poison
        nc.clear_and_free_semaphores(list(self.sems.allocated().values()))
        nc.all_engine_barrier()

    tile.TileContext._drain_and_barrier = _drain_and_barrier
    tile.TileContext._drain_patch_installed = True


def _legalize_waits(nc):
    """This image's walrus accepts at most ONE sync wait per instruction.
    Hoist extra waits into preceding single-wait NOPs on the same engine
    (engines execute their stream in order, so a prior NOP-wait gates the
    instruction identically)."""
    import concourse.mybir as mybir

    n = 0
    for bass_bb in nc.bb_map.values():
        insts = bass_bb.bb.instructions
        i = 0
        while i < len(insts):
            inst = insts[i]
            si = inst.sync_info
            waits = list(si.on_wait) if si is not None and si.on_wait else []
            if len(waits) > 1:
                for w in waits[:-1]:
                    nop = mybir.InstNoOp(
                        name=f"waitnop_{n}", engine=inst.engine, ins=[], outs=[],
                        sync_info=mybir.SyncInfo(on_wait=[w], on_update=[]),
                    )
                    n += 1
                    insts.insert(i, nop)
                    i += 1
                si.on_wait = waits[-1:]
            i += 1
    return n


def _d_segments(w0, w1):
    """(kk, s0, s1) pieces of dram-column range [w0, w1) split at component
    boundaries (784 columns per component)."""
    segs = []
    for kk in range(w0 // O, (w1 - 1) // O + 1):
        s0, s1 = max(w0, kk * O), min(w1, (kk + 1) * O)
        segs.append((kk, s0, s1))
    return segs


def _build_program():
    import concourse.bass as bass
    import concourse.mybir as mybir
    import concourse.tile as tile

    _install_drain_patch()
    f32 = mybir.dt.float32
    bf16 = mybir.dt.bfloat16
    f8 = mybir.dt.float8e4
    AF = mybir.ActivationFunctionType
    ALU = mybir.AluOpType
    DR = mybir.MatmulPerfMode.DoubleRow

    nc = bass.Bass("TRN2", target_bir_lowering=False, debug=False,
                   num_devices=N_CORES)

    d_xT = nc.dram_tensor("xT", [IC, 128, T], bf16, kind="ExternalInput").ap()
    d_wi2hT = nc.dram_tensor("wi2hT", [IC, 128, HID], bf16,
                             kind="ExternalInput").ap()
    d_bi16 = nc.dram_tensor("bi16", [1, HID], bf16, kind="ExternalInput").ap()
    d_wzT = nc.dram_tensor("wzT", [128, JC, K], bf16, kind="ExternalInput").ap()
    d_bz = nc.dram_tensor("bz", [1, K], f32, kind="ExternalInput").ap()
    d_wdT = nc.dram_tensor("wdT", [128, JC, R], f8, kind="ExternalInput").ap()
    d_bd256 = nc.dram_tensor("bd256", [1, R], bf16, kind="ExternalInput").ap()
    d_bdb = nc.dram_tensor("bdb", [128, R], bf16, kind="ExternalInput").ap()
    d_out = nc.dram_tensor("out", [T, O], f32, kind="ExternalOutput").ap()

    with tile.TileContext(nc) as tc:
        with (
            tc.tile_pool(name="consts", bufs=1) as consts,
            tc.tile_pool(name="hpool", bufs=1) as hpool,
            tc.tile_pool(name="upool", bufs=1) as upool,
            tc.tile_pool(name="epool", bufs=1) as epool,
            tc.tile_pool(name="tmp", bufs=2) as tmp,
            tc.tile_pool(name="hz_psum", bufs=1, space="PSUM") as hz_psum,
            tc.tile_pool(name="z_psum", bufs=1, space="PSUM") as z_psum,
            tc.tile_pool(name="d_psum", bufs=3, space="PSUM") as d_psum,
            tc.tile_pool(name="wslab", bufs=8) as wslab_pool,
            tc.tile_pool(name="bslab", bufs=3) as bslab_pool,
            tc.tile_pool(name="dtmp", bufs=3) as dtmp,
        ):

            # ---- phase H: h8[j, t] = fp8(16*leaky_relu(x W^T + b)) ---------
            h8_sb = hpool.tile([128, JC, T], f8, tag="h8", name="h8")
            esc_sb = [None] * TCHUNKS

            def load_slabs(w):
                w0 = WIN_OFF[w]
                win = WIN_PLAN[w]
                wsls = []
                for sub in range(0, win, 512):
                    sw = min(512, win - sub)
                    wsl = wslab_pool.tile([128, JC, sw], f8, tag="w",
                                          name=f"wsl{w}_{sub}")
                    for ja in range(0, JC, 2):
                        nc.sync.dma_start(
                            wsl[:, ja:ja + 2, :],
                            d_wdT[:, ja:ja + 2, w0 + sub:w0 + sub + sw])
                    wsls.append((sub, sw, wsl))
                if w % PE_BIAS_EVERY:
                    bsl = bslab_pool.tile([128, win], bf16, tag="b",
                                          name=f"bsl{w}")
                    half = win // 2
                    nc.scalar.dma_start(bsl[:, 0:half], d_bdb[:, w0:w0 + half])
                    nc.scalar.dma_start(bsl[:, half:win],
                                        d_bdb[:, w0 + half:w0 + win])
                else:
                    bsl = None
                return wsls, bsl

            with (
                tc.tile_pool(name="xw", bufs=1) as xw,
            ):
                x_sb, wi_sb = [], []
                for i in range(IC):
                    xt = xw.tile([128, T], bf16, tag=f"x{i}", name=f"x{i}")
                    x_sb.append(xt)
                    wt = xw.tile([128, HID], bf16, tag=f"wi{i}", name=f"wi{i}")
                    wi_sb.append(wt)
                # split the loads so the first matmul's operands land first
                # (one dma_start = one HW queue; fine pieces spread queues)
                for i in range(IC):
                    nc.sync.dma_start(wi_sb[i][:, 0:128], d_wi2hT[i][:, 0:128])
                    nc.scalar.dma_start(x_sb[i][:, 0:256], d_xT[i][:, 0:256])
                    nc.sync.dma_start(x_sb[i][:, 256:512], d_xT[i][:, 256:512])
                # constants on the otherwise-idle vector/gpsimd DMA queues so
                # the first d-window rank-1 (needs bd) isn't gated on the
                # scalar queue backlog
                bi16_sb = consts.tile([1, HID], bf16)
                nc.gpsimd.dma_start(bi16_sb[:], d_bi16[:])
                bd_sb = consts.tile([1, R], bf16)
                nc.gpsimd.dma_start(bd_sb[:, 0:R // 2], d_bd256[:, 0:R // 2])
                nc.gpsimd.dma_start(bd_sb[:, R // 2:R], d_bd256[:, R // 2:R])
                wz_sb = consts.tile([128, JC, K], bf16)
                nc.gpsimd.dma_start(wz_sb[:], d_wzT[:])
                bz_sb = consts.tile([1, K], f32)
                nc.gpsimd.dma_start(bz_sb[:], d_bz[:])
                ones_sb = consts.tile([1, 128], f32)
                nc.vector.memset(ones_sb[:], 1.0)
                onesb_sb = consts.tile([1, 128], bf16)
                nc.vector.memset(onesb_sb[:], 1.0)
                onest_sb = consts.tile([1, 512], bf16)
                nc.vector.memset(onest_sb[:], 1.0)
                u_sb = []
                for t in range(TCHUNKS):
                    u = upool.tile([128, O], f32, tag=f"u{t}", name=f"u{t}")
                    nc.vector.memset(u[:], 0.0)
                    u_sb.append(u)
                for i in range(IC):
                    for n4, (c0, c1) in enumerate(((128, 512), (512, 768),
                                                   (768, HID))):
                        eng = nc.scalar if n4 % 2 else nc.sync
                        eng.dma_start(wi_sb[i][:, c0:c1], d_wi2hT[i][:, c0:c1])
                    if T > 512:
                        nc.scalar.dma_start(x_sb[i][:, 512:T],
                                            d_xT[i][:, 512:T])
                preloaded = {w: load_slabs(w) for w in range(4)}

                # H and Z interleaved: after each 512-token half of h is
                # done, immediately compute that half's softmax numerators
                for tw in range(T // 512):
                    for j in range(JC):
                        ph = hz_psum.tile([128, 512], f32, tag="ph")
                        # bias via rank-1: b_row[j-chunk] x ones_tokens
                        nc.tensor.matmul(
                            ph[:],
                            lhsT=bi16_sb[:, j * 128:(j + 1) * 128],
                            rhs=onest_sb[:],
                            start=True,
                            stop=False,
                        )
                        for i in range(IC):
                            nc.tensor.matmul(
                                ph[:],
                                lhsT=wi_sb[i][:, j * 128:(j + 1) * 128],
                                rhs=x_sb[i][:, tw * 512:(tw + 1) * 512],
                                start=False,
                                stop=(i == IC - 1),
                            )
                        # 16*leaky_relu = 0.01*ph + 0.99*relu(ph); split so the
                        # DVE stt reads only one PSUM operand (HW restriction)
                        r1 = xw.tile([128, 512], f32, tag="r1", bufs=2,
                                     name=f"r1_{tw}_{j}")
                        nc.scalar.activation(r1[:], ph[:], AF.Relu, scale=0.99)
                        nc.vector.scalar_tensor_tensor(
                            out=h8_sb[:, j, tw * 512:(tw + 1) * 512],
                            in0=ph[:], scalar=0.01, in1=r1[:],
                            op0=ALU.mult, op1=ALU.add,
                        )
                    for t in range(tw * 4, tw * 4 + 4):
                        pz = z_psum.tile([128, K], f32, tag="pz",
                                          name=f"pz{t}")
                        for j in range(JC):
                            # fp8 stationary x bf16 moving is legal
                            nc.tensor.matmul(
                                pz[:],
                                lhsT=h8_sb[:, j, t * 128:(t + 1) * 128],
                                rhs=wz_sb[:, j, :],
                                start=(j == 0),
                                stop=False,
                            )
                        # + b_h2z via rank-1 update: ones[t] x bz
                        nc.tensor.matmul(
                            pz[:],
                            lhsT=ones_sb[:],
                            rhs=bz_sb[:],
                            start=False,
                            stop=True,
                        )
                        e_t = epool.tile([128, K], f32, tag=f"e{t}",
                                         name=f"e{t}")
                        s_t = tmp.tile([128, 1], f32, tag="s", name=f"s{t}")
                        nc.scalar.activation(e_t[:], pz[:], AF.Exp,
                                             accum_out=s_t[:])
                        sinv = tmp.tile([128, 1], f32, tag="sinv",
                                        name=f"sinv{t}")
                        nc.vector.reciprocal(sinv[:], s_t[:])
                        esc = epool.tile([128, K], f32, tag=f"esc{t}",
                                         name=f"esc{t}")
                        nc.vector.tensor_scalar(esc[:], e_t[:], sinv[:], None,
                                                ALU.mult)
                        esc_sb[t] = esc

            # ---- phase D: stream W shard, accumulate U ---------------------
            for w in range(N_WIN):
                w0 = WIN_OFF[w]
                win = WIN_PLAN[w]
                w1 = w0 + win
                wsls, bsl = preloaded.pop(w) if w in preloaded else load_slabs(w)
                segs = _d_segments(w0, w1)
                pe_bias = w % PE_BIAS_EVERY == 0
                t_order = range(TCHUNKS)
                if w == N_WIN - 1:
                    t_order = reversed(range(TCHUNKS))
                for t in t_order:
                    pd = d_psum.tile([128, win], f32, tag="pd", name=f"pd{w}_{t}")
                    # j2 outer / sub inner: both 512-subs reuse the same
                    # stationary h tile
                    if pe_bias:
                        for sub, sw, wsl in wsls:
                            nc.tensor.matmul(
                                pd[:, sub:sub + sw],
                                lhsT=onesb_sb[:],
                                rhs=bd_sb[:, w0 + sub:w0 + sub + sw],
                                start=True,
                                stop=False,
                            )
                    for j2 in range(JC2):
                        for sub, sw, wsl in wsls:
                            nc.tensor.matmul(
                                pd[:, sub:sub + sw],
                                lhsT=h8_sb[:, 2 * j2:2 * j2 + 2,
                                           t * 128:(t + 1) * 128],
                                rhs=wsl[:, 2 * j2:2 * j2 + 2, :],
                                start=(not pe_bias and j2 == 0),
                                stop=(j2 == JC2 - 1),
                                perf_mode=DR,
                            )
                    ds = dtmp.tile([128, win], f32, tag="ds")
                    if pe_bias:
                        nc.scalar.activation(ds[:], pd[:], AF.Sigmoid,
                                             scale=DSCALE)
                    else:
                        db = dtmp.tile([128, win], f32, tag="db")
                        nc.vector.scalar_tensor_tensor(
                            out=db[:], in0=pd[:], scalar=DSCALE, in1=bsl[:],
                            op0=ALU.mult, op1=ALU.add,
                        )
                        nc.scalar.activation(ds[:], db[:], AF.Sigmoid)
                    for kk, s0, s1 in segs:
                        nc.vector.scalar_tensor_tensor(
                            out=u_sb[t][:, s0 - kk * O:s1 - kk * O],
                            in0=ds[:, s0 - w0:s1 - w0],
                            scalar=esc_sb[t][:, kk:kk + 1],
                            in1=u_sb[t][:, s0 - kk * O:s1 - kk * O],
                            op0=ALU.mult, op1=ALU.add,
                        )

            for t in reversed(range(TCHUNKS)):
                nc.scalar.dma_start(d_out[t * 128:(t + 1) * 128, 0:392],
                                  u_sb[t][:, 0:392])
                nc.scalar.dma_start(d_out[t * 128:(t + 1) * 128, 392:O],
                                  u_sb[t][:, 392:O])

    _legalize_waits(nc)
    return nc


def _get_program():
    global _PROGRAM
    if _PROGRAM is None:
        _PROGRAM = _build_program()
    return _PROGRAM


def _prep_inputs(input, w_i2h, b_i2h, w_h2z, b_h2z, w_h2d, b_h2d):
    """Build the 8 per-core in_maps (host-side transposes/shards)."""
    import ml_dtypes
    x_flat = np.ascontiguousarray(input.reshape(B * L, IN).astype(np.float32))
    # W_i2h pre-scaled 16x (exact in bf16): h is carried at 16x everywhere
    wi2hT = np.ascontiguousarray(
        HSCALE * w_i2h.astype(np.float32).T.reshape(IC, 128, HID)
    ).astype(ml_dtypes.bfloat16)
    bi16 = np.ascontiguousarray(
        (HSCALE * b_i2h.astype(np.float32)).reshape(1, HID)
    ).astype(ml_dtypes.bfloat16)

    # w_h2z pre-scaled 1/16: (16h) @ (wz/16) keeps softmax logits exact
    wzT_full = w_h2z.astype(np.float32).T / HSCALE   # [HID, K]
    bz = np.ascontiguousarray(b_h2z.astype(np.float32).reshape(1, K))
    wz = np.ascontiguousarray(
        wzT_full.reshape(JC, 128, K).transpose(1, 0, 2)
    ).astype(ml_dtypes.bfloat16)

    # w_h2d at 16x in e4m3 (TRN FP8_EXP4: same bits as OCP e4m3 below
    # |240|; 16*w stays ~N(0, 0.25) so everything is in normal range)
    wdT_full = w_h2d.astype(np.float32).T            # [HID, R]
    wd = np.ascontiguousarray(
        (WSCALE * wdT_full).reshape(JC, 128, R).transpose(1, 0, 2)
    ).astype(ml_dtypes.float8_e4m3)
    # d-bias at 256x (compensated by sigmoid's 1/256 input scale) for the
    # PE rank-1 path, and a broadcast unscaled copy for the DVE stt path
    bd_f32 = b_h2d.astype(np.float32)
    bd256 = np.ascontiguousarray(
        (bd_f32 / DSCALE).reshape(1, R)).astype(ml_dtypes.bfloat16)
    bdb = np.ascontiguousarray(np.broadcast_to(
        bd_f32.astype(ml_dtypes.bfloat16), (128, R)))

    in_maps = []
    for core in range(N_CORES):
        xT = np.ascontiguousarray(
            x_flat[core * T:(core + 1) * T, :].T.reshape(IC, 128, T)
        ).astype(ml_dtypes.bfloat16)
        in_maps.append({
            "xT": xT, "wi2hT": wi2hT, "bi16": bi16,
            "wzT": wz, "bz": bz, "wdT": wd, "bd256": bd256, "bdb": bdb,
        })
    return in_maps


LAST_RESULT = None


def kernel(**inputs):
    from concourse.bass_utils import run_bass_kernel_spmd

    global LAST_RESULT
    nc = _get_program()
    in_maps = _prep_inputs(**inputs)
    trace = bool(os.environ.get("BASS_KERNEL_TRACE"))
    if trace:
        try:
            _install_profile_shim()
        except Exception as e:  # degrade to untraced run
            print(f"profile shim unavailable ({e}); running untraced")
            trace = False
    res = run_bass_kernel_spmd(nc, in_maps, list(range(N_CORES)), trace=trace)
    LAST_RESULT = res

    out = np.empty((B * L, O), dtype=np.float32)
    for tg in range(TOK_GROUPS):
        out[tg * T:(tg + 1) * T] = res.results[tg]["out"].astype(np.float32)
    return out.reshape(B, L, O)


def _install_profile_shim():
    """Register the NTFF profile hook concourse expects under axon (the
    image's antenv lacks axon_hooks) and stub the artifact upload."""
    import sys
    import types

    if "antenv.axon_hooks" not in sys.modules:
        from trn_agent_boot.trn_boot import _ntff_profile_via_ctypes

        hook = _ntff_profile_via_ctypes("/opt/axon/libaxon_pjrt.so")
        m = types.ModuleType("antenv.axon_hooks")
        m.get_axon_ntff_profile_hook = lambda: hook
        m.set_axon_ntff_profile_hook = lambda h: None
        sys.modules["antenv.axon_hooks"] = m

    import concourse.bass_utils as bu

    bu.upload_artifacts = lambda tmpdir: f"local://{tmpdir}"
